# revision 36
# baseline (speedup 1.0000x reference)
"""Multi-head causal self-attention (B=2, T=2048, D=2048, 16 heads, RoPE)
on 8 Trainium2 NeuronCores.

Sharding strategy
-----------------
* Tensor-parallel over heads: each core owns 2 of the 16 heads for both
  batch elements. Each core reads the full (host-transposed, bf16) x
  and only its slice of qkv_w, and computes q/k in a transposed
  [head_dim, t] layout so RoPE and the score matmuls need no on-device
  transposes; v lands in natural [t, head_dim] layout.
* All matmul operands are bf16 (fp32 PSUM accumulation) - same PE
  column rate as fp32r but half the DMA / LDWEIGHTS / DVE cost.
* x and the weights are host-tiled so every SBUF load is one large
  contiguous DMA ([128, blocks-along-free] layout); the x pool holds
  2.5 chunks of prefetch so the projection never waits on HBM.
* Both batches' q/k/v stay resident in SBUF so the attention loop runs
  head-major: head 0 for both batches, fire its AllToAll, head 1 for
  both batches, fire the second - each A2A (1 MiB bf16) overlaps the
  remaining compute.
* Attention: sT[tk, tq] blocks on the PE, exp on the scalar engine,
  probabilities bf16, softmax denominator accumulated on the DVE and
  reduced with a ones-matmul; attention output lands directly in
  oT[dv, tq] layout, normalized via a rank-1 broadcast matmul. The v
  bias is folded out of the projection (sum_k p_k (v+b) = o + den*b)
  into a per-partition add on the normalized output.
* Output projection is data-parallel over rows (512 rows/core) in two
  passes: even head-tiles (gated on A2A #0 only, so it runs while head
  1 is still computing) parked in SBUF as bf16, odd head-tiles after
  A2A #1, combined on the DVE. wo prefetches into the SBUF freed by
  the qkv weights during attention.
"""

import numpy as np

B = 2
T = 2048
D = 2048
H = 16             # global heads
HD = 128           # head dim
NCORES = 8
HPC = H // NCORES  # heads per core (2)
W = HPC * HD       # per-core q/k/v feature width (256)
NKT = D // 128     # contraction tiles over the embedding dim (16)
TCH = 512          # t-chunk width
SCALE = 1.0 / np.sqrt(HD)

_CACHE = {}


def _build_module(t_total=T):
    import concourse.bacc as bacc
    import concourse.mybir as mybir
    import concourse.tile as tile

    F32 = mybir.dt.float32
    BF16 = mybir.dt.bfloat16
    ADD = mybir.AluOpType.add
    MULT = mybir.AluOpType.mult
    AF = mybir.ActivationFunctionType

    t_ch = t_total // TCH          # chunks per batch element (4)
    rows = B * t_total             # 4096
    rpc = rows // NCORES           # output rows per core (512)
    n_rt = rpc // 128              # row tiles per core (4)
    n_fc = D // TCH                # feature chunks of out proj (4)

    nc = bacc.Bacc("TRN2", target_bir_lowering=False, debug=False,
                   num_devices=NCORES)

    # ---- I/O (all big operands bf16; biases that ride ACT stay f32) ----
    # xT is host-tiled: [b, j, :, kt*TCH:(kt+1)*TCH] is x[b]^T's
    # [kt*128:(kt+1)*128, j*TCH:(j+1)*TCH] block, so chunk loads are 4
    # big contiguous DMAs instead of 16 strided ones. The weights are
    # host-reshaped the same way ([128, NKT*W], block kt at column kt*W).
    xT = nc.dram_tensor("xT", [B, t_ch, 128, NKT * TCH], BF16,
                        kind="ExternalInput")
    wq = nc.dram_tensor("wq", [128, NKT * W], BF16, kind="ExternalInput")
    wk = nc.dram_tensor("wk", [128, NKT * W], BF16, kind="ExternalInput")
    wv = nc.dram_tensor("wv", [128, NKT * W], BF16, kind="ExternalInput")
    bqk = nc.dram_tensor("bqk", [HD, 2 * HPC], F32, kind="ExternalInput")
    bv = nc.dram_tensor("bv", [HD, HPC], F32, kind="ExternalInput")
    wo = nc.dram_tensor("wo", [D, D], BF16, kind="ExternalInput")
    bo = nc.dram_tensor("bo", [1, D], BF16, kind="ExternalInput")
    cosT = nc.dram_tensor("cosT", [HD, t_total], BF16, kind="ExternalInput")
    sinT = nc.dram_tensor("sinT", [HD, t_total], BF16, kind="ExternalInput")
    pt = nc.dram_tensor("pt", [HD, HD], BF16, kind="ExternalInput")
    maskT = nc.dram_tensor("maskT", [HD, HD], BF16, kind="ExternalInput")
    onec = nc.dram_tensor("onec", [HD, 1], BF16, kind="ExternalInput")
    oner = nc.dram_tensor("oner", [1, HD], BF16, kind="ExternalInput")
    onerf = nc.dram_tensor("onerf", [1, HD], F32, kind="ExternalInput")
    y = nc.dram_tensor("y", [rpc, D], BF16, kind="ExternalOutput")

    with tile.TileContext(nc) as tc:
        frees = []

        def single(shape, dtype, name, flist=frees):
            t, free = tc.tile(shape, dtype, name=name)
            flist.append(free)
            return t

        # ---- constants resident in SBUF ----
        cos_sb = single([HD, t_total], BF16, "cos_sb")
        sin_sb = single([HD, t_total], BF16, "sin_sb")
        pt_sb = single([HD, HD], BF16, "pt_sb")
        mask_sb = single([HD, HD], BF16, "mask_sb")
        onec_sb = single([HD, 1], BF16, "onec_sb")
        oner_sb = single([1, HD], BF16, "oner_sb")
        onerf_sb = single([1, HD], F32, "onerf_sb")
        bqk_sb = single([HD, 2 * HPC], F32, "bqk_sb")
        bv_sb = single([HD, HPC], F32, "bv_sb")
        bo_sb = single([1, D], BF16, "bo_sb")

        q_st = [[single([128, t_total], BF16, f"q_st{b}{h}")
                 for h in range(HPC)] for b in range(B)]
        k_st = [[single([128, t_total], BF16, f"k_st{b}{h}")
                 for h in range(HPC)] for b in range(B)]
        # v blocks: column block (tt*HPC + h)*HD holds the
        # [t_local=128, dv=128] tile for time-tile tt, head h.
        v_all = [single([128, NKT * HPC * HD], BF16, f"v_all{b}")
                 for b in range(B)]

        # ---- DRAM bounce buffers for the per-head AllToAlls ----
        with tc.tile_pool(name="dram", bufs=1, space="DRAM") as dram:
            bounce_in = [dram.tile([NCORES * HD, rpc], BF16,
                                   name=f"bounce_in{h}") for h in range(HPC)]
            bounce_out = [dram.tile([NCORES * HD, rpc], BF16,
                                    name=f"bounce_out{h}") for h in range(HPC)]

            # PSUM pools: 8 banks total. One deep 4-slot pool serves the
            # qkv projection groups, the score tiles and the out-proj
            # accumulators; ot is double-buffered so the normalization
            # chain of unit i doesn't stall unit i+1's accumulation; rot,
            # den and the den-broadcast share a 2-slot pool.
            with tc.tile_pool(name="mm_ps", bufs=4, space="PSUM") as mm_ps, \
                 tc.tile_pool(name="misc_ps", bufs=2, space="PSUM") as misc_ps, \
                 tc.tile_pool(name="ot_ps", bufs=2, space="PSUM") as ot_ps:
                qk_ps = mm_ps
                st_ps = mm_ps
                rot_ps = misc_ps
                den_ps = misc_ps
                v_ps = qk_ps

                with tc.tile_pool(name="xt", bufs=10) as xt_pool, \
                     tc.tile_pool(name="tmp", bufs=6) as tmp_pool, \
                     tc.tile_pool(name="et", bufs=8) as et_pool, \
                     tc.tile_pool(name="nrm", bufs=3) as nrm_pool, \
                     tc.tile_pool(name="ets", bufs=2) as ets_pool, \
                     tc.tile_pool(name="oto", bufs=6) as oto_pool:

                    wfrees = []
                    wq_sb = single([128, NKT * W], BF16, "wq_sb", wfrees)
                    wk_sb = single([128, NKT * W], BF16, "wk_sb", wfrees)
                    wv_sb = single([128, NKT * W], BF16, "wv_sb", wfrees)

                    def attn_unit(b, h, c):
                        ot = ot_ps.tile([128, TCH], F32,
                                        name=f"ot{b}{h}{c}", tag="ot")
                        den = den_ps.tile([1, TCH], F32,
                                          name=f"den{b}{h}{c}", tag="misc")
                        ets = ets_pool.tile([128, TCH], BF16,
                                            name=f"ets{b}{h}{c}", tag="ets")
                        kmax = 4 * c + 3
                        for k in range(kmax + 1):
                            off = max(0, (k - 4 * c) * 128)
                            ksl = slice(k * 128, (k + 1) * 128)
                            st = st_ps.tile([128, TCH], F32,
                                            name=f"st{b}{h}{c}{k}", tag="mm")
                            q0 = c * TCH
                            nc.tensor.matmul(
                                st[:, off:TCH],
                                k_st[b][h][:, ksl],
                                q_st[b][h][:, q0 + off:q0 + TCH],
                                start=True, stop=True,
                                skip_group_check=True)
                            et = et_pool.tile([128, TCH], BF16,
                                              name=f"et{b}{h}{c}{k}",
                                              tag="et")
                            nc.scalar.activation(
                                et[:, off:TCH], st[:, off:TCH],
                                AF.Exp, bias=0.0, scale=float(SCALE))
                            if k >= 4 * c:
                                # zero the not-yet-causal triangle
                                nc.vector.tensor_tensor(
                                    et[:, off:off + 128],
                                    et[:, off:off + 128],
                                    mask_sb[:], MULT)
                            # denominator partials on the DVE
                            if k == 0:
                                nc.vector.tensor_copy(ets[:], et[:])
                            else:
                                nc.vector.tensor_tensor(
                                    ets[:, off:TCH], ets[:, off:TCH],
                                    et[:, off:TCH], ADD)
                            nc.tensor.matmul(
                                ot[:, off:TCH],
                                v_all[b][:, (k * HPC + h) * HD:
                                         (k * HPC + h + 1) * HD],
                                et[:, off:TCH],
                                start=(k == 0), stop=(k == kmax),
                                skip_group_check=True)
                        nc.tensor.matmul(
                            den[0:1, :], onec_sb[:], ets[:],
                            start=True, stop=True, skip_group_check=True)
                        # normalize by the softmax denominator
                        rc = nrm_pool.tile([1, TCH], F32,
                                           name=f"rc{b}{h}{c}", tag="rc")
                        rscr = nrm_pool.tile([1, TCH], F32,
                                             name=f"rscr{b}{h}{c}", tag="rc")
                        nc.vector.reciprocal_approx_accurate(
                            rc[:], den[0:1, :], rscr[:])
                        bc = rot_ps.tile([128, TCH], F32,
                                         name=f"bc{b}{h}{c}", tag="misc")
                        nc.tensor.matmul(bc[:], onerf_sb[:], rc[:],
                                         start=True, stop=True,
                                         skip_group_check=True)
                        bcs = nrm_pool.tile([128, TCH], BF16,
                                            name=f"bcs{b}{h}{c}", tag="bcs")
                        nc.scalar.copy(bcs[:], bc[:])
                        otn = oto_pool.tile([128, TCH], BF16,
                                            name=f"otn{b}{h}{c}", tag="otn")
                        nc.vector.tensor_tensor(otn[:], ot[:], bcs[:], MULT)
                        # deferred per-partition v bias
                        nc.vector.tensor_scalar_add(
                            otn[:], otn[:], bv_sb[:, h:h + 1])
                        # chunk (b, c) is row-block b*4+c
                        r = b * t_ch + c
                        nc.sync.dma_start(
                            bounce_in[h][r * HD:(r + 1) * HD, :], otn[:])

                    # ============ Phase 1: QKV + RoPE (both batches) ====
                    for b in range(B):
                        for j in range(t_ch):
                            tr = slice(j * TCH, (j + 1) * TCH)
                            # first iteration: interleave weight/const DMAs
                            # with the x-chunk groups so the first matmul
                            # group is gated on as little DMA as possible
                            # (wq + x group 0), and each later need lands
                            # just in time.
                            if b == 0 and j == 0:
                                qw = NKT * W // 4
                                for p in range(4):
                                    nc.sync.dma_start(
                                        wq_sb[:, p * qw:(p + 1) * qw],
                                        wq.ap()[:, p * qw:(p + 1) * qw])
                            xg = []
                            for g in range(4):
                                xtile = xt_pool.tile([128, 4 * TCH], BF16,
                                                     name=f"xg{b}{j}_{g}",
                                                     tag="xt")
                                nc.sync.dma_start(
                                    xtile[:],
                                    xT.ap()[b, j, :,
                                            g * 4 * TCH:(g + 1) * 4 * TCH])
                                xg.append(xtile)
                                if b == 0 and j == 0:
                                    if g == 0:
                                        nc.sync.dma_start(pt_sb[:],
                                                          pt.ap()[:, :])
                                        nc.sync.dma_start(bqk_sb[:],
                                                          bqk.ap()[:, :])
                                    elif g == 1:
                                        nc.sync.dma_start(cos_sb[:],
                                                          cosT.ap()[:, :])
                                        nc.sync.dma_start(sin_sb[:],
                                                          sinT.ap()[:, :])
                                    elif g == 2:
                                        nc.sync.dma_start(wk_sb[:],
                                                          wk.ap()[:, :])

                            def xmov(kt):
                                c0 = (kt % 4) * TCH
                                return xg[kt // 4][:, c0:c0 + TCH]

                            def xstat(kt, ts):
                                c0 = (kt % 4) * TCH + ts * 128
                                return xg[kt // 4][:, c0:c0 + 128]

                            if b == 0 and j == 0:
                                nc.sync.dma_start(wv_sb[:], wv.ap()[:, :])
                                nc.sync.dma_start(bv_sb[:], bv.ap()[:, :])
                                nc.sync.dma_start(mask_sb[:], maskT.ap()[:, :])
                                nc.sync.dma_start(onec_sb[:], onec.ap()[:, :])
                                nc.sync.dma_start(oner_sb[:], oner.ap()[:, :])
                                nc.sync.dma_start(onerf_sb[:], onerf.ap()[:, :])
                                nc.sync.dma_start(bo_sb[:], bo.ap()[:, :])

                            for which, w_sb, store in (
                                ("q", wq_sb, q_st[b]), ("k", wk_sb, k_st[b])):
                                for h in range(HPC):
                                    ps = qk_ps.tile([128, TCH], F32,
                                                    name=f"{which}ps{b}{j}{h}",
                                                    tag="mm")
                                    for kt in range(NKT):
                                        col = kt * W + h * HD
                                        nc.tensor.matmul(
                                            ps[:],
                                            w_sb[:, col:col + HD],
                                            xmov(kt),
                                            start=(kt == 0),
                                            stop=(kt == NKT - 1))
                                    # bias (per-partition) + round to bf16
                                    bcol = h if which == "q" else HPC + h
                                    qtmp = tmp_pool.tile(
                                        [128, TCH], BF16,
                                        name=f"{which}t{b}{j}{h}", tag="tmp")
                                    nc.scalar.activation(
                                        qtmp[:], ps[:], AF.Identity,
                                        bias=bqk_sb[:, bcol:bcol + 1],
                                        scale=1.0)
                                    # rotate-half via permutation matmul
                                    rp = rot_ps.tile([128, TCH], F32,
                                                     name=f"rp{b}{j}{h}",
                                                     tag="misc")
                                    nc.tensor.matmul(rp[:], pt_sb[:], qtmp[:],
                                                     start=True, stop=True)
                                    t1 = tmp_pool.tile([128, TCH], BF16,
                                                       name=f"t1_{b}{j}{h}",
                                                       tag="tmp")
                                    nc.vector.tensor_tensor(
                                        t1[:], qtmp[:], cos_sb[:, tr], MULT)
                                    t2 = tmp_pool.tile([128, TCH], BF16,
                                                       name=f"t2_{b}{j}{h}",
                                                       tag="tmp")
                                    nc.vector.tensor_tensor(
                                        t2[:], rp[:], sin_sb[:, tr], MULT)
                                    nc.vector.tensor_tensor(
                                        store[h][:, tr], t1[:], t2[:], ADD)

                            # v in natural [t, dv] layout, two t-tiles/psum.
                            # The v bias is NOT applied here: sum_k p_k
                            # (v_k + bv) = sum_k p_k v_k + den*bv, so after
                            # normalization it is a per-partition add on
                            # the attention output (see phase 2).
                            for half in range(2):
                                pv = v_ps.tile([128, TCH], F32,
                                               name=f"vps{b}{j}{half}",
                                               tag="mm")
                                for sub in range(2):
                                    ts = half * 2 + sub
                                    cs = sub * W
                                    for kt in range(NKT):
                                        nc.tensor.matmul(
                                            pv[:, cs:cs + W],
                                            xstat(kt, ts),
                                            wv_sb[:, kt * W:(kt + 1) * W],
                                            start=(kt == 0),
                                            stop=(kt == NKT - 1),
                                            skip_group_check=True)
                                # both (tt, h) blocks land contiguously
                                tt0 = j * 4 + half * 2
                                nc.vector.tensor_copy(
                                    v_all[b][:, tt0 * W:(tt0 + 2) * W],
                                    pv[:])
                            # batch 0's head-0 attention is ready: emit one
                            # unit per batch-1 chunk so the scheduler fills
                            # each side's pipeline bubbles with the other's
                            # matmuls.
                            if b == 1:
                                attn_unit(0, 0, j)

                    for f in reversed(wfrees):
                        f()

                    # wo / oc / os pools open for the whole attention phase
                    # so the out-projection weights prefetch into the SBUF
                    # freed by the qkv weights while attention runs.
                    with tc.tile_pool(name="oc", bufs=16) as oc_pool, \
                         tc.tile_pool(name="wop", bufs=32) as wo_pool, \
                         tc.tile_pool(name="os", bufs=17) as os_pool, \
                         tc.tile_pool(name="ost", bufs=3) as ost_pool:

                        # global head-tile kt = HPC*s + hl for source core s
                        evens = [HPC * s for s in range(NCORES)]
                        odds = [HPC * s + 1 for s in range(NCORES)]

                        def load_wts(kts, fc, tagp):
                            out = {}
                            for kt in kts:
                                t_ = wo_pool.tile([128, TCH], BF16,
                                                  name=f"wo{tagp}{fc}_{kt}",
                                                  tag="wo")
                                nc.sync.dma_start(
                                    t_[:],
                                    wo.ap()[kt * 128:(kt + 1) * 128,
                                            fc * TCH:(fc + 1) * TCH])
                                out[kt] = t_
                            return out

                        # prefetch pass-A (evens) wo tiles right away
                        awts = [load_wts(evens, fc, "a")
                                for fc in range(n_fc)]

                        # ============ Phase 2: attention ============
                        # (0,0,*) units were interleaved into phase 1.
                        # (1,0,*) and (0,1,*) interleave with each other so
                        # consecutive units always have an independent
                        # neighbor; A2A #0 fires once head 0 is done and the
                        # pass-A projection then covers the (1,1,*) tail and
                        # the A2A #1 flight.
                        for c in range(t_ch):
                            attn_unit(1, 0, c)
                            attn_unit(0, 1, c)
                        nc.gpsimd.collective_compute(
                            "AllToAll",
                            mybir.AluOpType.bypass,
                            replica_groups=[list(range(NCORES))],
                            ins=[bounce_in[0][:].opt()],
                            outs=[bounce_out[0][:].opt()],
                        )
                        for c in range(t_ch):
                            attn_unit(1, 1, c)
                        nc.gpsimd.collective_compute(
                            "AllToAll",
                            mybir.AluOpType.bypass,
                            replica_groups=[list(range(NCORES))],
                            ins=[bounce_in[1][:].opt()],
                            outs=[bounce_out[1][:].opt()],
                        )

                        # ============ Phase 4: output projection ========
                        oc = [None] * NKT

                        def load_oc(hl):
                            for s in range(NCORES):
                                kt = HPC * s + hl
                                t_ = oc_pool.tile([128, rpc], BF16,
                                                  name=f"oc{kt}", tag="oc")
                                nc.sync.dma_start(
                                    t_[:],
                                    bounce_out[hl][s * 128:(s + 1) * 128, :])
                                oc[kt] = t_

                        # Pass A: bias + even head-tiles for ALL out tiles
                        # - gated only on AllToAll #0, so it fills the PE
                        # while head 1 / AllToAll #1 are still in flight.
                        load_oc(0)
                        osp = {}
                        for fc in range(n_fc):
                            wts = awts[fc]
                            for rt in range(n_rt):
                                po = st_ps.tile([128, TCH], F32,
                                                name=f"po{fc}{rt}", tag="mm")
                                nc.tensor.matmul(
                                    po[:], oner_sb[:],
                                    bo_sb[0:1, fc * TCH:(fc + 1) * TCH],
                                    start=True, stop=False,
                                    skip_group_check=True)
                                for i, kt in enumerate(evens):
                                    nc.tensor.matmul(
                                        po[:],
                                        oc[kt][:, rt * 128:(rt + 1) * 128],
                                        wts[kt][:],
                                        start=False, stop=(i == NCORES - 1),
                                        skip_group_check=True)
                                p_ = os_pool.tile([128, TCH], BF16,
                                                  name=f"osp{fc}{rt}",
                                                  tag="osp")
                                nc.scalar.copy(p_[:], po[:])
                                osp[fc, rt] = p_
                        # Pass B: odd head-tiles (gated on AllToAll #1),
                        # combined with the parked evens on the DVE.
                        load_oc(1)
                        for fc in range(n_fc):
                            wts = load_wts(odds, fc, "b")
                            for rt in range(n_rt):
                                po = st_ps.tile([128, TCH], F32,
                                                name=f"po2_{fc}{rt}",
                                                tag="mm")
                                for i, kt in enumerate(odds):
                                    nc.tensor.matmul(
                                        po[:],
                                        oc[kt][:, rt * 128:(rt + 1) * 128],
                                        wts[kt][:],
                                        start=(i == 0),
                                        stop=(i == NCORES - 1),
                                        skip_group_check=True)
                                os_t = ost_pool.tile([128, TCH], BF16,
                                                    name=f"os{fc}{rt}",
                                                    tag="ost")
                                nc.vector.tensor_tensor(
                                    os_t[:], po[:], osp[fc, rt][:], ADD)
                                nc.sync.dma_start(
                                    y.ap()[rt * 128:(rt + 1) * 128,
                                           fc * TCH:(fc + 1) * TCH],
                                    os_t[:])

        for f in reversed(frees):
            f()

    nc.compile()
    return nc


def _host_inputs(x, qkv_w, qkv_b, out_w, out_b, t_total=T):
    """Build the per-core input maps (all host-side layout shuffling)."""
    import ml_dtypes

    f32 = np.float32
    bf16 = ml_dtypes.bfloat16

    x = np.asarray(x, dtype=f32)
    qkv_w = np.asarray(qkv_w, dtype=f32)
    qkv_b = np.asarray(qkv_b, dtype=f32)
    out_w = np.asarray(out_w, dtype=f32)
    out_b = np.asarray(out_b, dtype=f32)

    t_ch = t_total // TCH
    # host-tiled xT: [B, t_ch, 128, NKT*TCH], block kt at column kt*TCH
    xT = (x.transpose(0, 2, 1)
          .reshape(B, NKT, 128, t_ch, TCH)
          .transpose(0, 3, 2, 1, 4)
          .reshape(B, t_ch, 128, NKT * TCH)).astype(bf16)
    xT = np.ascontiguousarray(xT)
    qkv_wT = np.ascontiguousarray(qkv_w.T)                   # [D, 3D] f32

    def wtile(wslice):
        # [D, W] -> [128, NKT*W] with block kt at column kt*W
        return np.ascontiguousarray(
            wslice.reshape(NKT, 128, W).transpose(1, 0, 2)
            .reshape(128, NKT * W)).astype(bf16)
    wo_h = np.ascontiguousarray(out_w.T).astype(bf16)        # [D, D]
    bo_h = out_b.reshape(1, D).astype(bf16)

    half = HD // 2
    freq = (1.0 / (10000.0 ** (np.arange(half, dtype=np.float64) / half)))
    ang = freq[:, None] * np.arange(t_total, dtype=np.float64)[None, :]
    cos_h = np.cos(ang)
    sin_h = np.sin(ang)
    cosT = np.concatenate([cos_h, cos_h], axis=0).astype(bf16)
    sinT = np.concatenate([sin_h, sin_h], axis=0).astype(bf16)

    P = np.zeros((HD, HD), dtype=f32)
    P[np.arange(half), np.arange(half) + half] = -1.0
    P[np.arange(half) + half, np.arange(half)] = 1.0
    pt_h = np.ascontiguousarray(P.T).astype(bf16)

    mask = np.where(np.arange(HD)[:, None] > np.arange(HD)[None, :],
                    f32(0.0), f32(1.0)).astype(bf16)
    onec_h = np.ones((HD, 1), dtype=bf16)
    oner_h = np.ones((1, HD), dtype=bf16)

    in_maps = []
    for c in range(NCORES):
        g0 = c * W                 # first feature col of this core's heads
        wq_c = wtile(qkv_wT[:, g0:g0 + W])
        wk_c = wtile(qkv_wT[:, D + g0:D + g0 + W])
        wv_c = wtile(qkv_wT[:, 2 * D + g0:2 * D + g0 + W])
        bq_c = qkv_b[g0:g0 + W].reshape(HPC, HD).T          # [HD, HPC]
        bk_c = qkv_b[D + g0:D + g0 + W].reshape(HPC, HD).T
        bqk_c = np.concatenate([bq_c, bk_c], axis=1)        # [HD, 2*HPC]
        bv_c = qkv_b[2 * D + g0:2 * D + g0 + W].reshape(HPC, HD).T
        in_maps.append({
            "xT": xT, "wq": wq_c, "wk": wk_c, "wv": wv_c,
            "bqk": np.ascontiguousarray(bqk_c).astype(f32),
            "bv": np.ascontiguousarray(bv_c).astype(f32),
            "wo": wo_h, "bo": bo_h, "cosT": cosT, "sinT": sinT,
            "pt": pt_h, "maskT": mask,
            "onec": onec_h, "oner": oner_h,
            "onerf": np.ones((1, HD), dtype=f32),
        })
    return in_maps


def kernel(x, qkv_w, qkv_b, out_w, out_b):
    from concourse.bass_utils import run_bass_kernel_spmd

    if "nc" not in _CACHE:
        _CACHE["nc"] = _build_module()
    nc = _CACHE["nc"]

    in_maps = _host_inputs(x, qkv_w, qkv_b, out_w, out_b)
    res = run_bass_kernel_spmd(nc, in_maps, core_ids=list(range(NCORES)))
    y = np.concatenate([np.asarray(res.results[c]["y"], dtype=np.float32)
                        for c in range(NCORES)], axis=0)
    return y.reshape(B, T, D)


# revision 37
# speedup vs baseline: 1.0065x; 1.0065x over previous
"""Multi-head causal self-attention (B=2, T=2048, D=2048, 16 heads, RoPE)
on 8 Trainium2 NeuronCores.

Sharding strategy
-----------------
* Tensor-parallel over heads: each core owns 2 of the 16 heads for both
  batch elements. Each core reads the full (host-transposed, bf16) x
  and only its slice of qkv_w, and computes q/k in a transposed
  [head_dim, t] layout so RoPE and the score matmuls need no on-device
  transposes; v lands in natural [t, head_dim] layout.
* All matmul operands are bf16 (fp32 PSUM accumulation) - same PE
  column rate as fp32r but half the DMA / LDWEIGHTS / DVE cost.
* x and the weights are host-tiled so every SBUF load is one large
  contiguous DMA ([128, blocks-along-free] layout); the x pool holds
  2.5 chunks of prefetch so the projection never waits on HBM.
* Both batches' q/k/v stay resident in SBUF so the attention loop runs
  head-major: head 0 for both batches, fire its AllToAll, head 1 for
  both batches, fire the second - each A2A (1 MiB bf16) overlaps the
  remaining compute.
* Attention: sT[tk, tq] blocks on the PE, exp on the scalar engine,
  probabilities bf16, softmax denominator accumulated on the DVE and
  reduced with a ones-matmul; attention output lands directly in
  oT[dv, tq] layout, normalized via a rank-1 broadcast matmul. The v
  bias is folded out of the projection (sum_k p_k (v+b) = o + den*b)
  into a per-partition add on the normalized output.
* Output projection is data-parallel over rows (512 rows/core) in two
  passes: even head-tiles (gated on A2A #0 only, so it runs while head
  1 is still computing) parked in SBUF as bf16, odd head-tiles after
  A2A #1, combined on the DVE. wo prefetches into the SBUF freed by
  the qkv weights during attention.
"""

import numpy as np

B = 2
T = 2048
D = 2048
H = 16             # global heads
HD = 128           # head dim
NCORES = 8
HPC = H // NCORES  # heads per core (2)
W = HPC * HD       # per-core q/k/v feature width (256)
NKT = D // 128     # contraction tiles over the embedding dim (16)
TCH = 512          # t-chunk width
SCALE = 1.0 / np.sqrt(HD)

_CACHE = {}


def _build_module(t_total=T):
    import concourse.bacc as bacc
    import concourse.mybir as mybir
    import concourse.tile as tile

    F32 = mybir.dt.float32
    BF16 = mybir.dt.bfloat16
    ADD = mybir.AluOpType.add
    MULT = mybir.AluOpType.mult
    AF = mybir.ActivationFunctionType

    t_ch = t_total // TCH          # chunks per batch element (4)
    rows = B * t_total             # 4096
    rpc = rows // NCORES           # output rows per core (512)
    n_rt = rpc // 128              # row tiles per core (4)
    n_fc = D // TCH                # feature chunks of out proj (4)

    nc = bacc.Bacc("TRN2", target_bir_lowering=False, debug=False,
                   num_devices=NCORES)

    # ---- I/O (all big operands bf16; biases that ride ACT stay f32) ----
    # xT is host-tiled: [b, j, :, kt*TCH:(kt+1)*TCH] is x[b]^T's
    # [kt*128:(kt+1)*128, j*TCH:(j+1)*TCH] block, so chunk loads are 4
    # big contiguous DMAs instead of 16 strided ones. The weights are
    # host-reshaped the same way ([128, NKT*W], block kt at column kt*W).
    xT = nc.dram_tensor("xT", [B, t_ch, 128, NKT * TCH], BF16,
                        kind="ExternalInput")
    wq = nc.dram_tensor("wq", [128, NKT * W], BF16, kind="ExternalInput")
    wk = nc.dram_tensor("wk", [128, NKT * W], BF16, kind="ExternalInput")
    wv = nc.dram_tensor("wv", [128, NKT * W], BF16, kind="ExternalInput")
    bqk = nc.dram_tensor("bqk", [HD, 2 * HPC], F32, kind="ExternalInput")
    bv = nc.dram_tensor("bv", [HD, HPC], F32, kind="ExternalInput")
    wo = nc.dram_tensor("wo", [D, D], BF16, kind="ExternalInput")
    bo = nc.dram_tensor("bo", [1, D], BF16, kind="ExternalInput")
    cosT = nc.dram_tensor("cosT", [HD, t_total], BF16, kind="ExternalInput")
    sinT = nc.dram_tensor("sinT", [HD, t_total], BF16, kind="ExternalInput")
    pt = nc.dram_tensor("pt", [HD, HD], BF16, kind="ExternalInput")
    maskT = nc.dram_tensor("maskT", [HD, HD], BF16, kind="ExternalInput")
    onec = nc.dram_tensor("onec", [HD, 1], BF16, kind="ExternalInput")
    oner = nc.dram_tensor("oner", [1, HD], BF16, kind="ExternalInput")
    onerf = nc.dram_tensor("onerf", [1, HD], F32, kind="ExternalInput")
    y = nc.dram_tensor("y", [rpc, D], BF16, kind="ExternalOutput")

    with tile.TileContext(nc) as tc:
        frees = []

        def single(shape, dtype, name, flist=frees):
            t, free = tc.tile(shape, dtype, name=name)
            flist.append(free)
            return t

        # ---- constants resident in SBUF ----
        cos_sb = single([HD, t_total], BF16, "cos_sb")
        sin_sb = single([HD, t_total], BF16, "sin_sb")
        pt_sb = single([HD, HD], BF16, "pt_sb")
        mask_sb = single([HD, HD], BF16, "mask_sb")
        onec_sb = single([HD, 1], BF16, "onec_sb")
        oner_sb = single([1, HD], BF16, "oner_sb")
        onerf_sb = single([1, HD], F32, "onerf_sb")
        bqk_sb = single([HD, 2 * HPC], F32, "bqk_sb")
        bv_sb = single([HD, HPC], F32, "bv_sb")
        bo_sb = single([1, D], BF16, "bo_sb")

        q_st = [[single([128, t_total], BF16, f"q_st{b}{h}")
                 for h in range(HPC)] for b in range(B)]
        k_st = [[single([128, t_total], BF16, f"k_st{b}{h}")
                 for h in range(HPC)] for b in range(B)]
        # v blocks: column block (tt*HPC + h)*HD holds the
        # [t_local=128, dv=128] tile for time-tile tt, head h.
        v_all = [single([128, NKT * HPC * HD], BF16, f"v_all{b}")
                 for b in range(B)]

        # ---- DRAM bounce buffers for the per-head AllToAlls ----
        with tc.tile_pool(name="dram", bufs=1, space="DRAM") as dram:
            bounce_in = [dram.tile([NCORES * HD, rpc], BF16,
                                   name=f"bounce_in{h}") for h in range(HPC)]
            bounce_out = [dram.tile([NCORES * HD, rpc], BF16,
                                    name=f"bounce_out{h}") for h in range(HPC)]

            # PSUM pools: 8 banks total. One deep 4-slot pool serves the
            # qkv projection groups, the score tiles and the out-proj
            # accumulators; ot is double-buffered so the normalization
            # chain of unit i doesn't stall unit i+1's accumulation; rot,
            # den and the den-broadcast share a 2-slot pool.
            with tc.tile_pool(name="mm_ps", bufs=4, space="PSUM") as mm_ps, \
                 tc.tile_pool(name="misc_ps", bufs=2, space="PSUM") as misc_ps, \
                 tc.tile_pool(name="ot_ps", bufs=2, space="PSUM") as ot_ps:
                qk_ps = mm_ps
                st_ps = mm_ps
                rot_ps = misc_ps
                den_ps = misc_ps
                v_ps = qk_ps

                with tc.tile_pool(name="xt", bufs=10) as xt_pool, \
                     tc.tile_pool(name="tmp", bufs=6) as tmp_pool, \
                     tc.tile_pool(name="et", bufs=8) as et_pool, \
                     tc.tile_pool(name="nrm", bufs=3) as nrm_pool, \
                     tc.tile_pool(name="ets", bufs=2) as ets_pool, \
                     tc.tile_pool(name="oto", bufs=6) as oto_pool:

                    wfrees = []
                    wq_sb = single([128, NKT * W], BF16, "wq_sb", wfrees)
                    wk_sb = single([128, NKT * W], BF16, "wk_sb", wfrees)
                    wv_sb = single([128, NKT * W], BF16, "wv_sb", wfrees)

                    def attn_unit(b, h, c):
                        ot = ot_ps.tile([128, TCH], F32,
                                        name=f"ot{b}{h}{c}", tag="ot")
                        den = den_ps.tile([1, TCH], F32,
                                          name=f"den{b}{h}{c}", tag="misc")
                        ets = ets_pool.tile([128, TCH], BF16,
                                            name=f"ets{b}{h}{c}", tag="ets")
                        kmax = 4 * c + 3
                        for k in range(kmax + 1):
                            off = max(0, (k - 4 * c) * 128)
                            ksl = slice(k * 128, (k + 1) * 128)
                            st = st_ps.tile([128, TCH], F32,
                                            name=f"st{b}{h}{c}{k}", tag="mm")
                            q0 = c * TCH
                            nc.tensor.matmul(
                                st[:, off:TCH],
                                k_st[b][h][:, ksl],
                                q_st[b][h][:, q0 + off:q0 + TCH],
                                start=True, stop=True,
                                skip_group_check=True)
                            et = et_pool.tile([128, TCH], BF16,
                                              name=f"et{b}{h}{c}{k}",
                                              tag="et")
                            nc.scalar.activation(
                                et[:, off:TCH], st[:, off:TCH],
                                AF.Exp, bias=0.0, scale=float(SCALE))
                            if k >= 4 * c:
                                # zero the not-yet-causal triangle
                                nc.vector.tensor_tensor(
                                    et[:, off:off + 128],
                                    et[:, off:off + 128],
                                    mask_sb[:], MULT)
                            # denominator partials on the DVE
                            if k == 0:
                                nc.vector.tensor_copy(ets[:], et[:])
                            else:
                                nc.vector.tensor_tensor(
                                    ets[:, off:TCH], ets[:, off:TCH],
                                    et[:, off:TCH], ADD)
                            nc.tensor.matmul(
                                ot[:, off:TCH],
                                v_all[b][:, (k * HPC + h) * HD:
                                         (k * HPC + h + 1) * HD],
                                et[:, off:TCH],
                                start=(k == 0), stop=(k == kmax),
                                skip_group_check=True)
                        nc.tensor.matmul(
                            den[0:1, :], onec_sb[:], ets[:],
                            start=True, stop=True, skip_group_check=True)
                        # normalize by the softmax denominator
                        rc = nrm_pool.tile([1, TCH], F32,
                                           name=f"rc{b}{h}{c}", tag="rc")
                        rscr = nrm_pool.tile([1, TCH], F32,
                                             name=f"rscr{b}{h}{c}", tag="rc")
                        nc.vector.reciprocal_approx_accurate(
                            rc[:], den[0:1, :], rscr[:])
                        bc = rot_ps.tile([128, TCH], F32,
                                         name=f"bc{b}{h}{c}", tag="misc")
                        nc.tensor.matmul(bc[:], onerf_sb[:], rc[:],
                                         start=True, stop=True,
                                         skip_group_check=True)
                        bcs = nrm_pool.tile([128, TCH], BF16,
                                            name=f"bcs{b}{h}{c}", tag="bcs")
                        nc.scalar.copy(bcs[:], bc[:])
                        otn = oto_pool.tile([128, TCH], BF16,
                                            name=f"otn{b}{h}{c}", tag="otn")
                        nc.vector.tensor_tensor(otn[:], ot[:], bcs[:], MULT)
                        # deferred per-partition v bias
                        nc.vector.tensor_scalar_add(
                            otn[:], otn[:], bv_sb[:, h:h + 1])
                        # chunk (b, c) is row-block b*4+c
                        r = b * t_ch + c
                        nc.sync.dma_start(
                            bounce_in[h][r * HD:(r + 1) * HD, :], otn[:])

                    # ============ Phase 1: QKV + RoPE (both batches) ====
                    for b in range(B):
                        for j in range(t_ch):
                            tr = slice(j * TCH, (j + 1) * TCH)
                            # first iteration: interleave weight/const DMAs
                            # with the x-chunk groups so the first matmul
                            # group is gated on as little DMA as possible
                            # (wq + x group 0), and each later need lands
                            # just in time.
                            if b == 0 and j == 0:
                                qw = NKT * W // 4
                                for p in range(4):
                                    nc.sync.dma_start(
                                        wq_sb[:, p * qw:(p + 1) * qw],
                                        wq.ap()[:, p * qw:(p + 1) * qw])
                            xg = []
                            for g in range(4):
                                xtile = xt_pool.tile([128, 4 * TCH], BF16,
                                                     name=f"xg{b}{j}_{g}",
                                                     tag="xt")
                                nc.sync.dma_start(
                                    xtile[:],
                                    xT.ap()[b, j, :,
                                            g * 4 * TCH:(g + 1) * 4 * TCH])
                                xg.append(xtile)
                                if b == 0 and j == 0:
                                    if g == 0:
                                        nc.sync.dma_start(pt_sb[:],
                                                          pt.ap()[:, :])
                                        nc.sync.dma_start(bqk_sb[:],
                                                          bqk.ap()[:, :])
                                    elif g == 1:
                                        nc.sync.dma_start(cos_sb[:],
                                                          cosT.ap()[:, :])
                                        nc.sync.dma_start(sin_sb[:],
                                                          sinT.ap()[:, :])
                                    elif g == 2:
                                        nc.sync.dma_start(wk_sb[:],
                                                          wk.ap()[:, :])

                            def xmov(kt):
                                c0 = (kt % 4) * TCH
                                return xg[kt // 4][:, c0:c0 + TCH]

                            def xstat(kt, ts):
                                c0 = (kt % 4) * TCH + ts * 128
                                return xg[kt // 4][:, c0:c0 + 128]

                            if b == 0 and j == 0:
                                nc.sync.dma_start(wv_sb[:], wv.ap()[:, :])
                                nc.sync.dma_start(bv_sb[:], bv.ap()[:, :])
                                nc.sync.dma_start(mask_sb[:], maskT.ap()[:, :])
                                nc.sync.dma_start(onec_sb[:], onec.ap()[:, :])
                                nc.sync.dma_start(oner_sb[:], oner.ap()[:, :])
                                nc.sync.dma_start(onerf_sb[:], onerf.ap()[:, :])
                                nc.sync.dma_start(bo_sb[:], bo.ap()[:, :])

                            for which, w_sb, store in (
                                ("q", wq_sb, q_st[b]), ("k", wk_sb, k_st[b])):
                                for h in range(HPC):
                                    ps = qk_ps.tile([128, TCH], F32,
                                                    name=f"{which}ps{b}{j}{h}",
                                                    tag="mm")
                                    for kt in range(NKT):
                                        col = kt * W + h * HD
                                        nc.tensor.matmul(
                                            ps[:],
                                            w_sb[:, col:col + HD],
                                            xmov(kt),
                                            start=(kt == 0),
                                            stop=(kt == NKT - 1))
                                    # bias (per-partition) + round to bf16
                                    bcol = h if which == "q" else HPC + h
                                    qtmp = tmp_pool.tile(
                                        [128, TCH], BF16,
                                        name=f"{which}t{b}{j}{h}", tag="tmp")
                                    nc.scalar.activation(
                                        qtmp[:], ps[:], AF.Identity,
                                        bias=bqk_sb[:, bcol:bcol + 1],
                                        scale=1.0)
                                    # rotate-half via permutation matmul
                                    rp = rot_ps.tile([128, TCH], F32,
                                                     name=f"rp{b}{j}{h}",
                                                     tag="misc")
                                    nc.tensor.matmul(rp[:], pt_sb[:], qtmp[:],
                                                     start=True, stop=True)
                                    t1 = tmp_pool.tile([128, TCH], BF16,
                                                       name=f"t1_{b}{j}{h}",
                                                       tag="tmp")
                                    nc.vector.tensor_tensor(
                                        t1[:], qtmp[:], cos_sb[:, tr], MULT)
                                    t2 = tmp_pool.tile([128, TCH], BF16,
                                                       name=f"t2_{b}{j}{h}",
                                                       tag="tmp")
                                    nc.vector.tensor_tensor(
                                        t2[:], rp[:], sin_sb[:, tr], MULT)
                                    nc.vector.tensor_tensor(
                                        store[h][:, tr], t1[:], t2[:], ADD)

                            # v in natural [t, dv] layout, two t-tiles/psum.
                            # The v bias is NOT applied here: sum_k p_k
                            # (v_k + bv) = sum_k p_k v_k + den*bv, so after
                            # normalization it is a per-partition add on
                            # the attention output (see phase 2).
                            for half in range(2):
                                pv = v_ps.tile([128, TCH], F32,
                                               name=f"vps{b}{j}{half}",
                                               tag="mm")
                                for sub in range(2):
                                    ts = half * 2 + sub
                                    cs = sub * W
                                    for kt in range(NKT):
                                        nc.tensor.matmul(
                                            pv[:, cs:cs + W],
                                            xstat(kt, ts),
                                            wv_sb[:, kt * W:(kt + 1) * W],
                                            start=(kt == 0),
                                            stop=(kt == NKT - 1),
                                            skip_group_check=True)
                                # both (tt, h) blocks land contiguously
                                tt0 = j * 4 + half * 2
                                nc.vector.tensor_copy(
                                    v_all[b][:, tt0 * W:(tt0 + 2) * W],
                                    pv[:])
                            # batch 0's head-0 attention is ready: emit one
                            # unit per batch-1 chunk so the scheduler fills
                            # each side's pipeline bubbles with the other's
                            # matmuls.
                            if b == 1:
                                attn_unit(0, 0, j)

                    for f in reversed(wfrees):
                        f()

                    # wo / oc / os pools open for the whole attention phase
                    # so the out-projection weights prefetch into the SBUF
                    # freed by the qkv weights while attention runs.
                    with tc.tile_pool(name="oc", bufs=16) as oc_pool, \
                         tc.tile_pool(name="wop", bufs=32) as wo_pool, \
                         tc.tile_pool(name="os", bufs=17) as os_pool, \
                         tc.tile_pool(name="ost", bufs=3) as ost_pool:

                        # global head-tile kt = HPC*s + hl for source core s
                        evens = [HPC * s for s in range(NCORES)]
                        odds = [HPC * s + 1 for s in range(NCORES)]

                        def load_wts(kts, fc, tagp):
                            out = {}
                            for kt in kts:
                                t_ = wo_pool.tile([128, TCH], BF16,
                                                  name=f"wo{tagp}{fc}_{kt}",
                                                  tag="wo")
                                nc.sync.dma_start(
                                    t_[:],
                                    wo.ap()[kt * 128:(kt + 1) * 128,
                                            fc * TCH:(fc + 1) * TCH])
                                out[kt] = t_
                            return out

                        # prefetch pass-A (evens) wo tiles right away
                        awts = [load_wts(evens, fc, "a")
                                for fc in range(n_fc)]

                        # ============ Phase 2: attention ============
                        # (0,0,*) units were interleaved into phase 1.
                        # Fire A2A #0 as early as possible (right after the
                        # last head-0 unit) so the pass-A projection can
                        # fill attention's pipeline bubbles from mid-phase.
                        for c in range(t_ch):
                            attn_unit(1, 0, c)
                        nc.gpsimd.collective_compute(
                            "AllToAll",
                            mybir.AluOpType.bypass,
                            replica_groups=[list(range(NCORES))],
                            ins=[bounce_in[0][:].opt()],
                            outs=[bounce_out[0][:].opt()],
                        )
                        for c in range(t_ch):
                            attn_unit(0, 1, c)
                        for c in range(t_ch):
                            attn_unit(1, 1, c)
                        nc.gpsimd.collective_compute(
                            "AllToAll",
                            mybir.AluOpType.bypass,
                            replica_groups=[list(range(NCORES))],
                            ins=[bounce_in[1][:].opt()],
                            outs=[bounce_out[1][:].opt()],
                        )

                        # ============ Phase 4: output projection ========
                        oc = [None] * NKT

                        def load_oc(hl):
                            for s in range(NCORES):
                                kt = HPC * s + hl
                                t_ = oc_pool.tile([128, rpc], BF16,
                                                  name=f"oc{kt}", tag="oc")
                                nc.sync.dma_start(
                                    t_[:],
                                    bounce_out[hl][s * 128:(s + 1) * 128, :])
                                oc[kt] = t_

                        # Pass A: bias + even head-tiles for ALL out tiles
                        # - gated only on AllToAll #0, so it fills the PE
                        # while head 1 / AllToAll #1 are still in flight.
                        load_oc(0)
                        osp = {}
                        for fc in range(n_fc):
                            wts = awts[fc]
                            for rt in range(n_rt):
                                po = st_ps.tile([128, TCH], F32,
                                                name=f"po{fc}{rt}", tag="mm")
                                nc.tensor.matmul(
                                    po[:], oner_sb[:],
                                    bo_sb[0:1, fc * TCH:(fc + 1) * TCH],
                                    start=True, stop=False,
                                    skip_group_check=True)
                                for i, kt in enumerate(evens):
                                    nc.tensor.matmul(
                                        po[:],
                                        oc[kt][:, rt * 128:(rt + 1) * 128],
                                        wts[kt][:],
                                        start=False, stop=(i == NCORES - 1),
                                        skip_group_check=True)
                                p_ = os_pool.tile([128, TCH], BF16,
                                                  name=f"osp{fc}{rt}",
                                                  tag="osp")
                                nc.scalar.copy(p_[:], po[:])
                                osp[fc, rt] = p_
                        # Pass B: odd head-tiles (gated on AllToAll #1),
                        # combined with the parked evens on the DVE.
                        load_oc(1)
                        for fc in range(n_fc):
                            wts = load_wts(odds, fc, "b")
                            for rt in range(n_rt):
                                po = st_ps.tile([128, TCH], F32,
                                                name=f"po2_{fc}{rt}",
                                                tag="mm")
                                for i, kt in enumerate(odds):
                                    nc.tensor.matmul(
                                        po[:],
                                        oc[kt][:, rt * 128:(rt + 1) * 128],
                                        wts[kt][:],
                                        start=(i == 0),
                                        stop=(i == NCORES - 1),
                                        skip_group_check=True)
                                os_t = ost_pool.tile([128, TCH], BF16,
                                                    name=f"os{fc}{rt}",
                                                    tag="ost")
                                nc.vector.tensor_tensor(
                                    os_t[:], po[:], osp[fc, rt][:], ADD)
                                nc.sync.dma_start(
                                    y.ap()[rt * 128:(rt + 1) * 128,
                                           fc * TCH:(fc + 1) * TCH],
                                    os_t[:])

        for f in reversed(frees):
            f()

    nc.compile()
    return nc


def _host_inputs(x, qkv_w, qkv_b, out_w, out_b, t_total=T):
    """Build the per-core input maps (all host-side layout shuffling)."""
    import ml_dtypes

    f32 = np.float32
    bf16 = ml_dtypes.bfloat16

    x = np.asarray(x, dtype=f32)
    qkv_w = np.asarray(qkv_w, dtype=f32)
    qkv_b = np.asarray(qkv_b, dtype=f32)
    out_w = np.asarray(out_w, dtype=f32)
    out_b = np.asarray(out_b, dtype=f32)

    t_ch = t_total // TCH
    # host-tiled xT: [B, t_ch, 128, NKT*TCH], block kt at column kt*TCH
    xT = (x.transpose(0, 2, 1)
          .reshape(B, NKT, 128, t_ch, TCH)
          .transpose(0, 3, 2, 1, 4)
          .reshape(B, t_ch, 128, NKT * TCH)).astype(bf16)
    xT = np.ascontiguousarray(xT)
    qkv_wT = np.ascontiguousarray(qkv_w.T)                   # [D, 3D] f32

    def wtile(wslice):
        # [D, W] -> [128, NKT*W] with block kt at column kt*W
        return np.ascontiguousarray(
            wslice.reshape(NKT, 128, W).transpose(1, 0, 2)
            .reshape(128, NKT * W)).astype(bf16)
    wo_h = np.ascontiguousarray(out_w.T).astype(bf16)        # [D, D]
    bo_h = out_b.reshape(1, D).astype(bf16)

    half = HD // 2
    freq = (1.0 / (10000.0 ** (np.arange(half, dtype=np.float64) / half)))
    ang = freq[:, None] * np.arange(t_total, dtype=np.float64)[None, :]
    cos_h = np.cos(ang)
    sin_h = np.sin(ang)
    cosT = np.concatenate([cos_h, cos_h], axis=0).astype(bf16)
    sinT = np.concatenate([sin_h, sin_h], axis=0).astype(bf16)

    P = np.zeros((HD, HD), dtype=f32)
    P[np.arange(half), np.arange(half) + half] = -1.0
    P[np.arange(half) + half, np.arange(half)] = 1.0
    pt_h = np.ascontiguousarray(P.T).astype(bf16)

    mask = np.where(np.arange(HD)[:, None] > np.arange(HD)[None, :],
                    f32(0.0), f32(1.0)).astype(bf16)
    onec_h = np.ones((HD, 1), dtype=bf16)
    oner_h = np.ones((1, HD), dtype=bf16)

    in_maps = []
    for c in range(NCORES):
        g0 = c * W                 # first feature col of this core's heads
        wq_c = wtile(qkv_wT[:, g0:g0 + W])
        wk_c = wtile(qkv_wT[:, D + g0:D + g0 + W])
        wv_c = wtile(qkv_wT[:, 2 * D + g0:2 * D + g0 + W])
        bq_c = qkv_b[g0:g0 + W].reshape(HPC, HD).T          # [HD, HPC]
        bk_c = qkv_b[D + g0:D + g0 + W].reshape(HPC, HD).T
        bqk_c = np.concatenate([bq_c, bk_c], axis=1)        # [HD, 2*HPC]
        bv_c = qkv_b[2 * D + g0:2 * D + g0 + W].reshape(HPC, HD).T
        in_maps.append({
            "xT": xT, "wq": wq_c, "wk": wk_c, "wv": wv_c,
            "bqk": np.ascontiguousarray(bqk_c).astype(f32),
            "bv": np.ascontiguousarray(bv_c).astype(f32),
            "wo": wo_h, "bo": bo_h, "cosT": cosT, "sinT": sinT,
            "pt": pt_h, "maskT": mask,
            "onec": onec_h, "oner": oner_h,
            "onerf": np.ones((1, HD), dtype=f32),
        })
    return in_maps


def kernel(x, qkv_w, qkv_b, out_w, out_b):
    from concourse.bass_utils import run_bass_kernel_spmd

    if "nc" not in _CACHE:
        _CACHE["nc"] = _build_module()
    nc = _CACHE["nc"]

    in_maps = _host_inputs(x, qkv_w, qkv_b, out_w, out_b)
    res = run_bass_kernel_spmd(nc, in_maps, core_ids=list(range(NCORES)))
    y = np.concatenate([np.asarray(res.results[c]["y"], dtype=np.float32)
                        for c in range(NCORES)], axis=0)
    return y.reshape(B, T, D)


# revision 38
# speedup vs baseline: 1.0103x; 1.0038x over previous
"""Multi-head causal self-attention (B=2, T=2048, D=2048, 16 heads, RoPE)
on 8 Trainium2 NeuronCores.

Sharding strategy
-----------------
* Tensor-parallel over heads: each core owns 2 of the 16 heads for both
  batch elements. Each core reads the full (host-transposed, bf16) x
  and only its slice of qkv_w, and computes q/k in a transposed
  [head_dim, t] layout so RoPE and the score matmuls need no on-device
  transposes; v lands in natural [t, head_dim] layout.
* All matmul operands are bf16 (fp32 PSUM accumulation) - same PE
  column rate as fp32r but half the DMA / LDWEIGHTS / DVE cost.
* x and the weights are host-tiled so every SBUF load is one large
  contiguous DMA ([128, blocks-along-free] layout); the x pool holds
  2.5 chunks of prefetch so the projection never waits on HBM.
* Both batches' q/k/v stay resident in SBUF so the attention loop runs
  head-major: head 0 for both batches, fire its AllToAll, head 1 for
  both batches, fire the second - each A2A (1 MiB bf16) overlaps the
  remaining compute.
* Attention: sT[tk, tq] blocks on the PE, exp on the scalar engine,
  probabilities bf16, softmax denominator accumulated on the DVE and
  reduced with a ones-matmul; attention output lands directly in
  oT[dv, tq] layout, normalized via a rank-1 broadcast matmul. The v
  bias is folded out of the projection (sum_k p_k (v+b) = o + den*b)
  into a per-partition add on the normalized output.
* Output projection is data-parallel over rows (512 rows/core) in two
  passes: even head-tiles (gated on A2A #0 only, so it runs while head
  1 is still computing) parked in SBUF as bf16, odd head-tiles after
  A2A #1, combined on the DVE. wo prefetches into the SBUF freed by
  the qkv weights during attention.
"""

import numpy as np

B = 2
T = 2048
D = 2048
H = 16             # global heads
HD = 128           # head dim
NCORES = 8
HPC = H // NCORES  # heads per core (2)
W = HPC * HD       # per-core q/k/v feature width (256)
NKT = D // 128     # contraction tiles over the embedding dim (16)
TCH = 512          # t-chunk width
SCALE = 1.0 / np.sqrt(HD)

_CACHE = {}


def _build_module(t_total=T):
    import concourse.bacc as bacc
    import concourse.mybir as mybir
    import concourse.tile as tile

    F32 = mybir.dt.float32
    BF16 = mybir.dt.bfloat16
    ADD = mybir.AluOpType.add
    MULT = mybir.AluOpType.mult
    AF = mybir.ActivationFunctionType

    t_ch = t_total // TCH          # chunks per batch element (4)
    rows = B * t_total             # 4096
    rpc = rows // NCORES           # output rows per core (512)
    n_rt = rpc // 128              # row tiles per core (4)
    n_fc = D // TCH                # feature chunks of out proj (4)

    nc = bacc.Bacc("TRN2", target_bir_lowering=False, debug=False,
                   num_devices=NCORES)

    # ---- I/O (all big operands bf16; biases that ride ACT stay f32) ----
    # xT is host-tiled: [b, j, :, kt*TCH:(kt+1)*TCH] is x[b]^T's
    # [kt*128:(kt+1)*128, j*TCH:(j+1)*TCH] block, so chunk loads are 4
    # big contiguous DMAs instead of 16 strided ones. The weights are
    # host-reshaped the same way ([128, NKT*W], block kt at column kt*W).
    xT = nc.dram_tensor("xT", [B, t_ch, 128, NKT * TCH], BF16,
                        kind="ExternalInput")
    wq = nc.dram_tensor("wq", [128, NKT * W], BF16, kind="ExternalInput")
    wk = nc.dram_tensor("wk", [128, NKT * W], BF16, kind="ExternalInput")
    wv = nc.dram_tensor("wv", [128, NKT * W], BF16, kind="ExternalInput")
    bqk = nc.dram_tensor("bqk", [HD, 2 * HPC], F32, kind="ExternalInput")
    bv = nc.dram_tensor("bv", [HD, HPC], F32, kind="ExternalInput")
    wo = nc.dram_tensor("wo", [D, D], BF16, kind="ExternalInput")
    bo = nc.dram_tensor("bo", [1, D], BF16, kind="ExternalInput")
    cosT = nc.dram_tensor("cosT", [HD, t_total], BF16, kind="ExternalInput")
    sinT = nc.dram_tensor("sinT", [HD, t_total], BF16, kind="ExternalInput")
    pt = nc.dram_tensor("pt", [HD, HD], BF16, kind="ExternalInput")
    maskT = nc.dram_tensor("maskT", [HD, HD], BF16, kind="ExternalInput")
    onec = nc.dram_tensor("onec", [HD, 1], BF16, kind="ExternalInput")
    oner = nc.dram_tensor("oner", [1, HD], BF16, kind="ExternalInput")
    onerf = nc.dram_tensor("onerf", [1, HD], F32, kind="ExternalInput")
    y = nc.dram_tensor("y", [rpc, D], BF16, kind="ExternalOutput")

    with tile.TileContext(nc) as tc:
        frees = []

        def single(shape, dtype, name, flist=frees):
            t, free = tc.tile(shape, dtype, name=name)
            flist.append(free)
            return t

        # ---- constants resident in SBUF ----
        cos_sb = single([HD, t_total], BF16, "cos_sb")
        sin_sb = single([HD, t_total], BF16, "sin_sb")
        pt_sb = single([HD, HD], BF16, "pt_sb")
        mask_sb = single([HD, HD], BF16, "mask_sb")
        onec_sb = single([HD, 1], BF16, "onec_sb")
        oner_sb = single([1, HD], BF16, "oner_sb")
        onerf_sb = single([1, HD], F32, "onerf_sb")
        bqk_sb = single([HD, 2 * HPC], F32, "bqk_sb")
        bv_sb = single([HD, HPC], F32, "bv_sb")
        bo_sb = single([1, D], BF16, "bo_sb")

        q_st = [[single([128, t_total], BF16, f"q_st{b}{h}")
                 for h in range(HPC)] for b in range(B)]
        k_st = [[single([128, t_total], BF16, f"k_st{b}{h}")
                 for h in range(HPC)] for b in range(B)]
        # v blocks: column block (tt*HPC + h)*HD holds the
        # [t_local=128, dv=128] tile for time-tile tt, head h.
        v_all = [single([128, NKT * HPC * HD], BF16, f"v_all{b}")
                 for b in range(B)]

        # ---- DRAM bounce buffers for the per-head AllToAlls ----
        with tc.tile_pool(name="dram", bufs=1, space="DRAM") as dram:
            bounce_in = [dram.tile([NCORES * HD, rpc], BF16,
                                   name=f"bounce_in{h}") for h in range(HPC)]
            bounce_out = [dram.tile([NCORES * HD, rpc], BF16,
                                    name=f"bounce_out{h}") for h in range(HPC)]

            # PSUM pools: 8 banks total. One deep 4-slot pool serves the
            # qkv projection groups, the score tiles and the out-proj
            # accumulators; ot is double-buffered so the normalization
            # chain of unit i doesn't stall unit i+1's accumulation; rot,
            # den and the den-broadcast share a 2-slot pool.
            with tc.tile_pool(name="mm_ps", bufs=4, space="PSUM") as mm_ps, \
                 tc.tile_pool(name="misc_ps", bufs=2, space="PSUM") as misc_ps, \
                 tc.tile_pool(name="ot_ps", bufs=2, space="PSUM") as ot_ps:
                qk_ps = mm_ps
                st_ps = mm_ps
                rot_ps = misc_ps
                den_ps = misc_ps
                v_ps = qk_ps

                with tc.tile_pool(name="xt", bufs=10) as xt_pool, \
                     tc.tile_pool(name="tmp", bufs=6) as tmp_pool, \
                     tc.tile_pool(name="et", bufs=9) as et_pool, \
                     tc.tile_pool(name="nrm", bufs=4) as nrm_pool, \
                     tc.tile_pool(name="ets", bufs=2) as ets_pool, \
                     tc.tile_pool(name="oto", bufs=6) as oto_pool:

                    wfrees = []
                    wq_sb = single([128, NKT * W], BF16, "wq_sb", wfrees)
                    wk_sb = single([128, NKT * W], BF16, "wk_sb", wfrees)
                    wv_sb = single([128, NKT * W], BF16, "wv_sb", wfrees)

                    def attn_unit(b, h, c):
                        ot = ot_ps.tile([128, TCH], F32,
                                        name=f"ot{b}{h}{c}", tag="ot")
                        den = den_ps.tile([1, TCH], F32,
                                          name=f"den{b}{h}{c}", tag="misc")
                        ets = ets_pool.tile([128, TCH], BF16,
                                            name=f"ets{b}{h}{c}", tag="ets")
                        kmax = 4 * c + 3
                        for k in range(kmax + 1):
                            off = max(0, (k - 4 * c) * 128)
                            ksl = slice(k * 128, (k + 1) * 128)
                            st = st_ps.tile([128, TCH], F32,
                                            name=f"st{b}{h}{c}{k}", tag="mm")
                            q0 = c * TCH
                            nc.tensor.matmul(
                                st[:, off:TCH],
                                k_st[b][h][:, ksl],
                                q_st[b][h][:, q0 + off:q0 + TCH],
                                start=True, stop=True,
                                skip_group_check=True)
                            et = et_pool.tile([128, TCH], BF16,
                                              name=f"et{b}{h}{c}{k}",
                                              tag="et")
                            nc.scalar.activation(
                                et[:, off:TCH], st[:, off:TCH],
                                AF.Exp, bias=0.0, scale=float(SCALE))
                            if k >= 4 * c:
                                # zero the not-yet-causal triangle
                                nc.vector.tensor_tensor(
                                    et[:, off:off + 128],
                                    et[:, off:off + 128],
                                    mask_sb[:], MULT)
                            # denominator partials on the DVE
                            if k == 0:
                                nc.vector.tensor_copy(ets[:], et[:])
                            else:
                                nc.vector.tensor_tensor(
                                    ets[:, off:TCH], ets[:, off:TCH],
                                    et[:, off:TCH], ADD)
                            nc.tensor.matmul(
                                ot[:, off:TCH],
                                v_all[b][:, (k * HPC + h) * HD:
                                         (k * HPC + h + 1) * HD],
                                et[:, off:TCH],
                                start=(k == 0), stop=(k == kmax),
                                skip_group_check=True)
                        nc.tensor.matmul(
                            den[0:1, :], onec_sb[:], ets[:],
                            start=True, stop=True, skip_group_check=True)
                        # normalize by the softmax denominator
                        rc = nrm_pool.tile([1, TCH], F32,
                                           name=f"rc{b}{h}{c}", tag="rc")
                        rscr = nrm_pool.tile([1, TCH], F32,
                                             name=f"rscr{b}{h}{c}", tag="rc")
                        nc.vector.reciprocal_approx_accurate(
                            rc[:], den[0:1, :], rscr[:])
                        bc = rot_ps.tile([128, TCH], F32,
                                         name=f"bc{b}{h}{c}", tag="misc")
                        nc.tensor.matmul(bc[:], onerf_sb[:], rc[:],
                                         start=True, stop=True,
                                         skip_group_check=True)
                        bcs = nrm_pool.tile([128, TCH], BF16,
                                            name=f"bcs{b}{h}{c}", tag="bcs")
                        nc.scalar.copy(bcs[:], bc[:])
                        otn = oto_pool.tile([128, TCH], BF16,
                                            name=f"otn{b}{h}{c}", tag="otn")
                        nc.vector.tensor_tensor(otn[:], ot[:], bcs[:], MULT)
                        # deferred per-partition v bias
                        nc.vector.tensor_scalar_add(
                            otn[:], otn[:], bv_sb[:, h:h + 1])
                        # chunk (b, c) is row-block b*4+c
                        r = b * t_ch + c
                        nc.sync.dma_start(
                            bounce_in[h][r * HD:(r + 1) * HD, :], otn[:])

                    # ============ Phase 1: QKV + RoPE (both batches) ====
                    for b in range(B):
                        for j in range(t_ch):
                            tr = slice(j * TCH, (j + 1) * TCH)
                            # first iteration: interleave weight/const DMAs
                            # with the x-chunk groups so the first matmul
                            # group is gated on as little DMA as possible
                            # (wq + x group 0), and each later need lands
                            # just in time.
                            if b == 0 and j == 0:
                                qw = NKT * W // 4
                                for p in range(4):
                                    nc.sync.dma_start(
                                        wq_sb[:, p * qw:(p + 1) * qw],
                                        wq.ap()[:, p * qw:(p + 1) * qw])
                            xg = []
                            for g in range(4):
                                xtile = xt_pool.tile([128, 4 * TCH], BF16,
                                                     name=f"xg{b}{j}_{g}",
                                                     tag="xt")
                                nc.sync.dma_start(
                                    xtile[:],
                                    xT.ap()[b, j, :,
                                            g * 4 * TCH:(g + 1) * 4 * TCH])
                                xg.append(xtile)
                                if b == 0 and j == 0:
                                    if g == 0:
                                        nc.sync.dma_start(pt_sb[:],
                                                          pt.ap()[:, :])
                                        nc.sync.dma_start(bqk_sb[:],
                                                          bqk.ap()[:, :])
                                    elif g == 1:
                                        nc.sync.dma_start(cos_sb[:],
                                                          cosT.ap()[:, :])
                                        nc.sync.dma_start(sin_sb[:],
                                                          sinT.ap()[:, :])
                                    elif g == 2:
                                        nc.sync.dma_start(wk_sb[:],
                                                          wk.ap()[:, :])

                            def xmov(kt):
                                c0 = (kt % 4) * TCH
                                return xg[kt // 4][:, c0:c0 + TCH]

                            def xstat(kt, ts):
                                c0 = (kt % 4) * TCH + ts * 128
                                return xg[kt // 4][:, c0:c0 + 128]

                            if b == 0 and j == 0:
                                nc.sync.dma_start(wv_sb[:], wv.ap()[:, :])
                                nc.sync.dma_start(bv_sb[:], bv.ap()[:, :])
                                nc.sync.dma_start(mask_sb[:], maskT.ap()[:, :])
                                nc.sync.dma_start(onec_sb[:], onec.ap()[:, :])
                                nc.sync.dma_start(oner_sb[:], oner.ap()[:, :])
                                nc.sync.dma_start(onerf_sb[:], onerf.ap()[:, :])
                                nc.sync.dma_start(bo_sb[:], bo.ap()[:, :])

                            for which, w_sb, store in (
                                ("q", wq_sb, q_st[b]), ("k", wk_sb, k_st[b])):
                                for h in range(HPC):
                                    ps = qk_ps.tile([128, TCH], F32,
                                                    name=f"{which}ps{b}{j}{h}",
                                                    tag="mm")
                                    for kt in range(NKT):
                                        col = kt * W + h * HD
                                        nc.tensor.matmul(
                                            ps[:],
                                            w_sb[:, col:col + HD],
                                            xmov(kt),
                                            start=(kt == 0),
                                            stop=(kt == NKT - 1))
                                    # bias (per-partition) + round to bf16
                                    bcol = h if which == "q" else HPC + h
                                    qtmp = tmp_pool.tile(
                                        [128, TCH], BF16,
                                        name=f"{which}t{b}{j}{h}", tag="tmp")
                                    nc.scalar.activation(
                                        qtmp[:], ps[:], AF.Identity,
                                        bias=bqk_sb[:, bcol:bcol + 1],
                                        scale=1.0)
                                    # rotate-half via permutation matmul
                                    rp = rot_ps.tile([128, TCH], F32,
                                                     name=f"rp{b}{j}{h}",
                                                     tag="misc")
                                    nc.tensor.matmul(rp[:], pt_sb[:], qtmp[:],
                                                     start=True, stop=True)
                                    t1 = tmp_pool.tile([128, TCH], BF16,
                                                       name=f"t1_{b}{j}{h}",
                                                       tag="tmp")
                                    nc.vector.tensor_tensor(
                                        t1[:], qtmp[:], cos_sb[:, tr], MULT)
                                    t2 = tmp_pool.tile([128, TCH], BF16,
                                                       name=f"t2_{b}{j}{h}",
                                                       tag="tmp")
                                    nc.vector.tensor_tensor(
                                        t2[:], rp[:], sin_sb[:, tr], MULT)
                                    nc.vector.tensor_tensor(
                                        store[h][:, tr], t1[:], t2[:], ADD)

                            # v in natural [t, dv] layout, two t-tiles/psum.
                            # The v bias is NOT applied here: sum_k p_k
                            # (v_k + bv) = sum_k p_k v_k + den*bv, so after
                            # normalization it is a per-partition add on
                            # the attention output (see phase 2).
                            for half in range(2):
                                pv = v_ps.tile([128, TCH], F32,
                                               name=f"vps{b}{j}{half}",
                                               tag="mm")
                                for sub in range(2):
                                    ts = half * 2 + sub
                                    cs = sub * W
                                    for kt in range(NKT):
                                        nc.tensor.matmul(
                                            pv[:, cs:cs + W],
                                            xstat(kt, ts),
                                            wv_sb[:, kt * W:(kt + 1) * W],
                                            start=(kt == 0),
                                            stop=(kt == NKT - 1),
                                            skip_group_check=True)
                                # both (tt, h) blocks land contiguously
                                tt0 = j * 4 + half * 2
                                nc.vector.tensor_copy(
                                    v_all[b][:, tt0 * W:(tt0 + 2) * W],
                                    pv[:])
                            # batch 0's head-0 attention is ready: emit one
                            # unit per batch-1 chunk so the scheduler fills
                            # each side's pipeline bubbles with the other's
                            # matmuls.
                            if b == 1:
                                attn_unit(0, 0, j)

                    for f in reversed(wfrees):
                        f()

                    # wo / oc / os pools open for the whole attention phase
                    # so the out-projection weights prefetch into the SBUF
                    # freed by the qkv weights while attention runs.
                    with tc.tile_pool(name="oc", bufs=16) as oc_pool, \
                         tc.tile_pool(name="wop", bufs=32) as wo_pool, \
                         tc.tile_pool(name="os", bufs=17) as os_pool, \
                         tc.tile_pool(name="ost", bufs=3) as ost_pool:

                        # global head-tile kt = HPC*s + hl for source core s
                        evens = [HPC * s for s in range(NCORES)]
                        odds = [HPC * s + 1 for s in range(NCORES)]

                        def load_wts(kts, fc, tagp):
                            out = {}
                            for kt in kts:
                                t_ = wo_pool.tile([128, TCH], BF16,
                                                  name=f"wo{tagp}{fc}_{kt}",
                                                  tag="wo")
                                nc.sync.dma_start(
                                    t_[:],
                                    wo.ap()[kt * 128:(kt + 1) * 128,
                                            fc * TCH:(fc + 1) * TCH])
                                out[kt] = t_
                            return out

                        # prefetch pass-A (evens) wo tiles right away
                        awts = [load_wts(evens, fc, "a")
                                for fc in range(n_fc)]

                        # ============ Phase 2: attention ============
                        # (0,0,*) units were interleaved into phase 1.
                        # Fire A2A #0 as early as possible (right after the
                        # last head-0 unit) so the pass-A projection can
                        # fill attention's pipeline bubbles from mid-phase.
                        for c in range(t_ch):
                            attn_unit(1, 0, c)
                        nc.gpsimd.collective_compute(
                            "AllToAll",
                            mybir.AluOpType.bypass,
                            replica_groups=[list(range(NCORES))],
                            ins=[bounce_in[0][:].opt()],
                            outs=[bounce_out[0][:].opt()],
                        )
                        for c in range(t_ch):
                            attn_unit(0, 1, c)
                        for c in range(t_ch):
                            attn_unit(1, 1, c)
                        nc.gpsimd.collective_compute(
                            "AllToAll",
                            mybir.AluOpType.bypass,
                            replica_groups=[list(range(NCORES))],
                            ins=[bounce_in[1][:].opt()],
                            outs=[bounce_out[1][:].opt()],
                        )

                        # ============ Phase 4: output projection ========
                        oc = [None] * NKT

                        def load_oc(hl):
                            for s in range(NCORES):
                                kt = HPC * s + hl
                                t_ = oc_pool.tile([128, rpc], BF16,
                                                  name=f"oc{kt}", tag="oc")
                                nc.sync.dma_start(
                                    t_[:],
                                    bounce_out[hl][s * 128:(s + 1) * 128, :])
                                oc[kt] = t_

                        # Pass A: bias + even head-tiles for ALL out tiles
                        # - gated only on AllToAll #0, so it fills the PE
                        # while head 1 / AllToAll #1 are still in flight.
                        load_oc(0)
                        osp = {}
                        for fc in range(n_fc):
                            wts = awts[fc]
                            for rt in range(n_rt):
                                po = st_ps.tile([128, TCH], F32,
                                                name=f"po{fc}{rt}", tag="mm")
                                nc.tensor.matmul(
                                    po[:], oner_sb[:],
                                    bo_sb[0:1, fc * TCH:(fc + 1) * TCH],
                                    start=True, stop=False,
                                    skip_group_check=True)
                                for i, kt in enumerate(evens):
                                    nc.tensor.matmul(
                                        po[:],
                                        oc[kt][:, rt * 128:(rt + 1) * 128],
                                        wts[kt][:],
                                        start=False, stop=(i == NCORES - 1),
                                        skip_group_check=True)
                                p_ = os_pool.tile([128, TCH], BF16,
                                                  name=f"osp{fc}{rt}",
                                                  tag="osp")
                                nc.scalar.copy(p_[:], po[:])
                                osp[fc, rt] = p_
                        # Pass B: odd head-tiles (gated on AllToAll #1),
                        # combined with the parked evens on the DVE.
                        load_oc(1)
                        for fc in range(n_fc):
                            wts = load_wts(odds, fc, "b")
                            for rt in range(n_rt):
                                po = st_ps.tile([128, TCH], F32,
                                                name=f"po2_{fc}{rt}",
                                                tag="mm")
                                for i, kt in enumerate(odds):
                                    nc.tensor.matmul(
                                        po[:],
                                        oc[kt][:, rt * 128:(rt + 1) * 128],
                                        wts[kt][:],
                                        start=(i == 0),
                                        stop=(i == NCORES - 1),
                                        skip_group_check=True)
                                os_t = ost_pool.tile([128, TCH], BF16,
                                                    name=f"os{fc}{rt}",
                                                    tag="ost")
                                nc.vector.tensor_tensor(
                                    os_t[:], po[:], osp[fc, rt][:], ADD)
                                nc.sync.dma_start(
                                    y.ap()[rt * 128:(rt + 1) * 128,
                                           fc * TCH:(fc + 1) * TCH],
                                    os_t[:])

        for f in reversed(frees):
            f()

    nc.compile()
    return nc


def _host_inputs(x, qkv_w, qkv_b, out_w, out_b, t_total=T):
    """Build the per-core input maps (all host-side layout shuffling)."""
    import ml_dtypes

    f32 = np.float32
    bf16 = ml_dtypes.bfloat16

    x = np.asarray(x, dtype=f32)
    qkv_w = np.asarray(qkv_w, dtype=f32)
    qkv_b = np.asarray(qkv_b, dtype=f32)
    out_w = np.asarray(out_w, dtype=f32)
    out_b = np.asarray(out_b, dtype=f32)

    t_ch = t_total // TCH
    # host-tiled xT: [B, t_ch, 128, NKT*TCH], block kt at column kt*TCH
    xT = (x.transpose(0, 2, 1)
          .reshape(B, NKT, 128, t_ch, TCH)
          .transpose(0, 3, 2, 1, 4)
          .reshape(B, t_ch, 128, NKT * TCH)).astype(bf16)
    xT = np.ascontiguousarray(xT)
    qkv_wT = np.ascontiguousarray(qkv_w.T)                   # [D, 3D] f32

    def wtile(wslice):
        # [D, W] -> [128, NKT*W] with block kt at column kt*W
        return np.ascontiguousarray(
            wslice.reshape(NKT, 128, W).transpose(1, 0, 2)
            .reshape(128, NKT * W)).astype(bf16)
    wo_h = np.ascontiguousarray(out_w.T).astype(bf16)        # [D, D]
    bo_h = out_b.reshape(1, D).astype(bf16)

    half = HD // 2
    freq = (1.0 / (10000.0 ** (np.arange(half, dtype=np.float64) / half)))
    ang = freq[:, None] * np.arange(t_total, dtype=np.float64)[None, :]
    cos_h = np.cos(ang)
    sin_h = np.sin(ang)
    cosT = np.concatenate([cos_h, cos_h], axis=0).astype(bf16)
    sinT = np.concatenate([sin_h, sin_h], axis=0).astype(bf16)

    P = np.zeros((HD, HD), dtype=f32)
    P[np.arange(half), np.arange(half) + half] = -1.0
    P[np.arange(half) + half, np.arange(half)] = 1.0
    pt_h = np.ascontiguousarray(P.T).astype(bf16)

    mask = np.where(np.arange(HD)[:, None] > np.arange(HD)[None, :],
                    f32(0.0), f32(1.0)).astype(bf16)
    onec_h = np.ones((HD, 1), dtype=bf16)
    oner_h = np.ones((1, HD), dtype=bf16)

    in_maps = []
    for c in range(NCORES):
        g0 = c * W                 # first feature col of this core's heads
        wq_c = wtile(qkv_wT[:, g0:g0 + W])
        wk_c = wtile(qkv_wT[:, D + g0:D + g0 + W])
        wv_c = wtile(qkv_wT[:, 2 * D + g0:2 * D + g0 + W])
        bq_c = qkv_b[g0:g0 + W].reshape(HPC, HD).T          # [HD, HPC]
        bk_c = qkv_b[D + g0:D + g0 + W].reshape(HPC, HD).T
        bqk_c = np.concatenate([bq_c, bk_c], axis=1)        # [HD, 2*HPC]
        bv_c = qkv_b[2 * D + g0:2 * D + g0 + W].reshape(HPC, HD).T
        in_maps.append({
            "xT": xT, "wq": wq_c, "wk": wk_c, "wv": wv_c,
            "bqk": np.ascontiguousarray(bqk_c).astype(f32),
            "bv": np.ascontiguousarray(bv_c).astype(f32),
            "wo": wo_h, "bo": bo_h, "cosT": cosT, "sinT": sinT,
            "pt": pt_h, "maskT": mask,
            "onec": onec_h, "oner": oner_h,
            "onerf": np.ones((1, HD), dtype=f32),
        })
    return in_maps


def kernel(x, qkv_w, qkv_b, out_w, out_b):
    from concourse.bass_utils import run_bass_kernel_spmd

    if "nc" not in _CACHE:
        _CACHE["nc"] = _build_module()
    nc = _CACHE["nc"]

    in_maps = _host_inputs(x, qkv_w, qkv_b, out_w, out_b)
    res = run_bass_kernel_spmd(nc, in_maps, core_ids=list(range(NCORES)))
    y = np.concatenate([np.asarray(res.results[c]["y"], dtype=np.float32)
                        for c in range(NCORES)], axis=0)
    return y.reshape(B, T, D)
